# revision 8
# baseline (speedup 1.0000x reference)
"""BiPointNetConv Trainium2 kernel: 3x (sign-binarize -> 1x1 conv -> sync-BN -> ReLU) -> max-pool over K.

Sharding: data-parallel over batch dim B (8 cores x 1 batch element each);
params replicated.  BN batch stats are computed EXACTLY via Gram matrices of
the binarized activations (G = [B;1][B;1]^T, c = B 1), allreduced across cores
(one tiny [65,65] AllReduce per layer), so conv outputs are binarized straight
out of PSUM and never materialized.

Point enumeration is k-major (idx = k*N + n) so the neighbor max-pool is a
contiguous binary tree.  Activations are packed [128 = 2 column-halves x 64ch].
"""

import os
import numpy as np
from concourse import bass, bacc, tile, mybir
from concourse.bass_utils import run_bass_kernel_spmd

DT = mybir.dt
F32 = DT.float32
BF16 = DT.bfloat16
ALU = mybir.AluOpType
ACTF = mybir.ActivationFunctionType

BN_EPS = 1e-5
NCORES = 8
K = 32          # neighbors
D = 64          # input channels
SIZES = [64, 64, 64, 128]

# packed host-constant vector columns ([128,1] each)
HC_COLS = ["gs1", "beta1", "s21", "gs2", "beta2", "s22",
           "gs3", "beta3", "s23", "r41", "rng1", "rnr1"]


def build_nc(NB=8, n_cores=NCORES):
    """NB = number of 128-point n-blocks per core (N = 128*NB points/core)."""
    N = 128 * NB                 # points per core (n dim)
    NPTS = N * K                 # (n,k) pairs per core
    NCHUNK = NPTS // 128         # 128-point chunks (= K*NB)
    NPAIR = NCHUNK // 2
    HALF_F = NPTS // 2           # free size of packed [128, HALF_F] arenas
    SLICE = HALF_F // K          # width of one k-slice inside a half
    NG = float(n_cores * NPTS)   # global number of (n,k) points
    NT = HALF_F // 512           # 512-col tiles per packed arena
    assert HALF_F % 512 == 0 and NB % 2 == 0 and NT % 4 == 0

    nc = bacc.Bacc("TRN2", target_bir_lowering=False, debug=False,
                   num_devices=n_cores)

    # ---------------- DRAM parameters ----------------
    x_dram = nc.dram_tensor("x", [NPTS, D], F32, kind="ExternalInput").ap()
    out_dram = nc.dram_tensor("out", [N, SIZES[3]], F32,
                              kind="ExternalOutput").ap()
    wfwd_d = [nc.dram_tensor(f"wfwd{l}", [128, SIZES[l]], BF16,
                             kind="ExternalInput").ap() for l in (1, 2, 3)]
    wstat_d = [nc.dram_tensor(f"wstat{l}", [64, 128], F32,
                              kind="ExternalInput").ap() for l in (1, 2, 3)]
    hc_d = nc.dram_tensor("hcpack", [128, len(HC_COLS)], F32,
                          kind="ExternalInput").ap()
    idb_d = nc.dram_tensor("idb", [128, 128], BF16, kind="ExternalInput").ap()
    idf_d = nc.dram_tensor("idf", [128, 128], F32, kind="ExternalInput").ap()
    ones_d = nc.dram_tensor("ones64", [64, 1], F32, kind="ExternalInput").ap()
    # collective bounce buffers
    cc_in = [nc.dram_tensor(f"cc_in{l}", [65, 65], F32) for l in (1, 2, 3)]
    cc_out = [nc.dram_tensor(f"cc_out{l}", [65, 65], F32,
                             addr_space="Shared") for l in (1, 2, 3)]

    # ---------------- persistent SBUF ----------------
    b1t = nc.alloc_sbuf_tensor("b1t", [128, 65 * NCHUNK], BF16).ap()
    arenaA = nc.alloc_sbuf_tensor("arenaA", [128, HALF_F], BF16).ap()  # b1, a3
    arenaB = nc.alloc_sbuf_tensor("arenaB", [128, HALF_F], BF16).ap()  # a2
    arenaT = nc.alloc_sbuf_tensor("arenaT", [128, 130 * NPAIR], BF16).ap()
    stgmax = nc.alloc_sbuf_tensor("stgmax", [128, 2 * 4 * SLICE], BF16).ap()
    scr = nc.alloc_sbuf_tensor("scr", [128, 4 * SLICE], BF16).ap()
    maxz = nc.alloc_sbuf_tensor("maxz", [128, 2 * SLICE], BF16).ap()
    y_sb = nc.alloc_sbuf_tensor("y_sb", [128, 128 * NB], F32).ap()
    yt_sb = nc.alloc_sbuf_tensor("yt_sb", [128, 128 * NB], F32).ap()

    wfwd = [nc.alloc_sbuf_tensor(f"wfwd{l}_sb", [128, SIZES[l]], BF16).ap()
            for l in (1, 2, 3)]
    wstat = [nc.alloc_sbuf_tensor(f"wstat{l}_sb", [64, 128], F32).ap()
             for l in (1, 2, 3)]
    idb = nc.alloc_sbuf_tensor("idb_sb", [128, 128], BF16).ap()
    idf = nc.alloc_sbuf_tensor("idf_sb", [128, 128], F32).ap()
    ones64 = nc.alloc_sbuf_tensor("ones_sb", [64, 1], F32).ap()
    hcp = nc.alloc_sbuf_tensor("hcp_sb", [128, len(HC_COLS)], F32).ap()

    def hc(nm):
        i = HC_COLS.index(nm)
        return hcp[:, i:i + 1]

    # small device-computed vectors, packed in one tensor
    VEC_COLS = []
    for l in (1, 2, 3):
        VEC_COLS += [f"{nm}{l}" for nm in
                     ("sz", "q", "t0", "t1", "m", "var", "istd", "u", "U", "V")]
    VEC_COLS += ["sgn3", "absu3", "w3"]
    vecs = nc.alloc_sbuf_tensor("vecs", [128, len(VEC_COLS)], F32).ap()

    def vv(nm):
        i = VEC_COLS.index(nm)
        return vecs[:, i:i + 1]

    gbuf = nc.alloc_sbuf_tensor("gbuf", [65, 65 * 6], F32).ap()

    def gpay(l):
        return gbuf[:, 65 * (l - 1):65 * l]

    def gglob(l):
        return gbuf[:, 65 * (2 + l):65 * (3 + l)]

    mbuf = nc.alloc_sbuf_tensor("mbuf", [64, 256], F32).ap()
    t1m, melem = mbuf[:, 0:128], mbuf[:, 128:256]

    with tile.TileContext(nc) as tc:
        with (tc.tile_pool(name="psum", bufs=3, space="PSUM") as pp,
              tc.tile_pool(name="psum_g", bufs=2, space="PSUM") as pg,
              tc.tile_pool(name="psum_tr", bufs=2, space="PSUM") as ptr):

            # ---------- const loads ----------
            for i in range(3):
                nc.sync.dma_start(wfwd[i][:], wfwd_d[i][:])
                nc.sync.dma_start(wstat[i][:], wstat_d[i][:])
            nc.sync.dma_start(idb[:], idb_d[:])
            nc.sync.dma_start(idf[:], idf_d[:])
            nc.sync.dma_start(ones64[:], ones_d[:])
            nc.sync.dma_start(hcp[:], hc_d[:])

            # ---------- ones columns in transposed arenas ----------
            b1tv = b1t.rearrange("p (c e) -> p c e", e=65)
            atv = arenaT.rearrange("p (g e) -> p g e", e=65)
            nc.vector.memset(b1tv[:, :, 64:65], 1.0)
            nc.vector.memset(atv[:, :, 64:65], 1.0)

            # ---------- load x per k-slice + binarize: b1t = (x>0) ----------
            # src row = n*K + k, n = 128*nb + p ; b1t chunk c = k*NB + nb
            xv = x_dram.rearrange("(nb p k) d -> p k nb d", p=128, k=K)
            with tc.tile_pool(name="xp", bufs=4) as xp:
                for k in range(K):
                    xt = xp.tile([128, NB * 64], F32, tag="x")
                    nc.sync.dma_start(xt[:], xv[:, k])
                    nc.vector.tensor_scalar(
                        b1tv[:, k * NB:(k + 1) * NB, 0:64],
                        xt.rearrange("p (nb d) -> p nb d", d=64),
                        0.0, None, op0=ALU.is_gt)

            def transpose_pairs(src_pair, dst, nP):
                """PE-transpose [128,128] pairs; dst(m0, w) gives dest AP."""
                for m0 in range(0, nP, 8):
                    w = 128 * min(8, nP - m0)
                    pt = ptr.tile([128, 1024], BF16, tag="tr")
                    for j in range(min(8, nP - m0)):
                        nc.tensor.transpose(pt[:, 128 * j:128 * (j + 1)],
                                            src_pair(m0 + j), idb[:])
                    nc.vector.tensor_copy(dst(m0, w), pt[:, :w])

            # ---------- b1 (channel-major packed) from b1t ----------
            # per-chunk [128,64] transposes: chunk 2m+h -> psum[64h:64h+64]
            for m0 in range(0, NPAIR, 8):
                nj = min(8, NPAIR - m0)
                pt = ptr.tile([128, 1024], BF16, tag="tr")
                for j in range(nj):
                    for h in range(2):
                        c = 2 * (m0 + j) + h
                        nc.tensor.transpose(
                            pt[64 * h:64 * h + 64, 128 * j:128 * (j + 1)],
                            b1t[:, 65 * c:65 * c + 64], idb[:],
                            tile_position=(0, 64 * h))
                nc.vector.tensor_copy(
                    arenaA[:, 128 * m0:128 * (m0 + nj)], pt[:, :128 * nj])

            # ---------- G1 over chunks ----------
            g1ps = pg.tile([65, 65], F32, tag="g")
            for c in range(NCHUNK):
                sl = b1t[:, 65 * c:65 * c + 65]
                nc.tensor.matmul(g1ps[:], sl, sl,
                                 start=(c == 0), stop=(c == NCHUNK - 1))
            nc.vector.tensor_copy(gpay(1), g1ps[:])

            # ---------- helpers ----------
            def allreduce_G(l):
                nc.gpsimd.dma_start(cc_in[l - 1][:], gpay(l))
                nc.gpsimd.collective_compute(
                    "AllReduce", ALU.add,
                    replica_groups=[list(range(n_cores))],
                    ins=[cc_in[l - 1][:]], outs=[cc_out[l - 1][:]])
                nc.gpsimd.dma_start(gglob(l), cc_out[l - 1][:])

            def stats_and_thresholds(l):
                i = l - 1
                G = gglob(l)[0:64, 0:64]
                ccol = gglob(l)[0:64, 64:65]
                # sz' = W c  -> dup'd [128,1]
                p1 = pg.tile([128, 1], F32, tag="g")
                nc.tensor.matmul(p1[:], wstat[i][:], ccol)
                nc.vector.tensor_copy(vv(f"sz{l}"), p1[:])
                # q' = diag(W G W^T): T1 = G @ Wt_dup ; melem = Wt*T1 ; q = 1^T melem
                p2 = pg.tile([64, 128], F32, tag="g")
                nc.tensor.matmul(p2[:], G, wstat[i][:])
                nc.vector.tensor_copy(t1m[:], p2[:])
                nc.vector.tensor_tensor(melem[:], t1m[:], wstat[i][:],
                                        op=ALU.mult)
                p3 = pg.tile([128, 1], F32, tag="g")
                nc.tensor.matmul(p3[:], melem[:], ones64[:])
                nc.vector.tensor_copy(vv(f"q{l}"), p3[:])

                sz, q = vv(f"sz{l}"), vv(f"q{l}")
                t0, t1v = vv(f"t0{l}"), vv(f"t1{l}")
                m, var = vv(f"m{l}"), vv(f"var{l}")
                istd, u = vv(f"istd{l}"), vv(f"u{l}")
                U, V = vv(f"U{l}"), vv(f"V{l}")
                if l == 1:
                    # +-1 encoding: z_true = 2z' - r
                    # sz_t = 2 sz - Ng*r   (rng1 = Ng*r)
                    # q_t  = 4q - 4r*sz + Ng*r^2   (r41 = 4r, rnr1 = Ng*r^2)
                    nc.vector.tensor_scalar(t0[:], sz[:], 2.0, None,
                                            op0=ALU.mult)
                    nc.vector.tensor_tensor(t0[:], t0[:], hc("rng1"),
                                            op=ALU.subtract)
                    nc.vector.tensor_scalar(t1v[:], q[:], 4.0, None,
                                            op0=ALU.mult)
                    nc.vector.tensor_tensor(q[:], hc("r41"), sz[:],
                                            op=ALU.mult)
                    nc.vector.tensor_tensor(t1v[:], t1v[:], q[:],
                                            op=ALU.subtract)
                    nc.vector.tensor_tensor(t1v[:], t1v[:], hc("rnr1"),
                                            op=ALU.add)
                    nc.vector.tensor_copy(sz[:], t0[:])
                    nc.vector.tensor_copy(q[:], t1v[:])
                nc.vector.tensor_scalar(m[:], sz[:], 1.0 / NG, None,
                                        op0=ALU.mult)
                nc.vector.tensor_scalar(var[:], q[:], 1.0 / NG, None,
                                        op0=ALU.mult)
                nc.vector.tensor_tensor(t0[:], m[:], m[:], op=ALU.mult)
                nc.vector.tensor_tensor(var[:], var[:], t0[:],
                                        op=ALU.subtract)
                # d = s2*var + eps ; istd = sqrt(1/d)
                nc.vector.tensor_tensor(t0[:], var[:], hc(f"s2{l}"),
                                        op=ALU.mult)
                nc.vector.tensor_scalar(t0[:], t0[:], BN_EPS, None,
                                        op0=ALU.add)
                nc.vector.reciprocal(t1v[:], t0[:])
                nc.scalar.activation(istd[:], t1v[:], ACTF.Sqrt)
                nc.vector.tensor_tensor(u[:], hc(f"gs{l}"), istd[:],
                                        op=ALU.mult)
                if l == 1:
                    nc.vector.tensor_scalar(U[:], u[:], 2.0, None,
                                            op0=ALU.mult)
                else:
                    nc.vector.tensor_copy(U[:], u[:])
                # V = u*m - beta (+ u*r for l=1)
                nc.vector.tensor_tensor(t0[:], u[:], m[:], op=ALU.mult)
                nc.vector.tensor_tensor(V[:], t0[:], hc(f"beta{l}"),
                                        op=ALU.subtract)
                if l == 1:
                    nc.vector.tensor_tensor(t0[:], u[:], hc("r41"),
                                            op=ALU.mult)
                    nc.vector.tensor_scalar(t0[:], t0[:], 0.25, None,
                                            op0=ALU.mult)
                    nc.vector.tensor_tensor(V[:], V[:], t0[:], op=ALU.add)
                if l == 3:
                    nc.scalar.activation(vv("sgn3"), u[:], ACTF.Sign)
                    nc.scalar.activation(vv("absu3"), u[:], ACTF.Abs)
                    nc.vector.tensor_scalar(vv("w3"), V[:], -1.0, None,
                                            op0=ALU.mult)

            def fwd_bin_layer(l, src, dst):
                """layers 1/2: z = W x (2-row-group packed); binarize to dst."""
                i = l - 1
                for t in range(NT):
                    zp = pp.tile([128, 512], F32, tag="z")
                    nc.tensor.matmul(zp[0:64, :], wfwd[i][0:64, :],
                                     src[0:64, 512 * t:512 * (t + 1)],
                                     tile_position=(0, 0))
                    nc.tensor.matmul(zp[64:128, :], wfwd[i][64:128, :],
                                     src[64:128, 512 * t:512 * (t + 1)],
                                     tile_position=(64, 64))
                    nc.vector.tensor_scalar(
                        dst[:, 512 * t:512 * (t + 1)], zp[:],
                        vv(f"U{l}"), vv(f"V{l}"),
                        op0=ALU.mult, op1=ALU.is_gt)

            def tree_max(src_flat, dst, n, width):
                """src_flat: [128, n*width]; pairwise-max halve until dst."""
                cur = src_flat
                while n > 1:
                    n //= 2
                    out = dst if n == 1 else scr[:, :n * width]
                    nc.vector.tensor_tensor(out, cur[:, :n * width],
                                            cur[:, n * width:2 * n * width],
                                            op=ALU.max)
                    cur = out

            # ================= layer 1 =================
            allreduce_G(1)
            stats_and_thresholds(1)
            fwd_bin_layer(1, arenaA, arenaB)

            # ---------- a2 transposes + G2 ----------
            def atv_dst(m0, w):
                return atv[:, 2 * m0:2 * m0 + w // 64, 0:64]

            transpose_pairs(lambda m: arenaB[:, 128 * m:128 * (m + 1)],
                            atv_dst, NPAIR)
            gA = pg.tile([65, 65], F32, tag="g")
            gB = pg.tile([65, 65], F32, tag="g")
            for g in range(NPAIR):
                nc.tensor.matmul(gA[:], arenaT[:, 130 * g:130 * g + 65],
                                 arenaT[:, 130 * g:130 * g + 65],
                                 start=(g == 0), stop=(g == NPAIR - 1))
                nc.tensor.matmul(gB[:], arenaT[:, 130 * g + 65:130 * g + 130],
                                 arenaT[:, 130 * g + 65:130 * g + 130],
                                 start=(g == 0), stop=(g == NPAIR - 1))
            nc.vector.tensor_copy(gpay(2), gA[:])
            nc.vector.tensor_tensor(gpay(2), gpay(2), gB[:], op=ALU.add)

            # ================= layer 2 =================
            allreduce_G(2)
            stats_and_thresholds(2)
            fwd_bin_layer(2, arenaB, arenaA)

            # ---------- a3 transposes + G3 ----------
            transpose_pairs(lambda m: arenaA[:, 128 * m:128 * (m + 1)],
                            atv_dst, NPAIR)
            gA3 = pg.tile([65, 65], F32, tag="g")
            gB3 = pg.tile([65, 65], F32, tag="g")
            for g in range(NPAIR):
                nc.tensor.matmul(gA3[:], arenaT[:, 130 * g:130 * g + 65],
                                 arenaT[:, 130 * g:130 * g + 65],
                                 start=(g == 0), stop=(g == NPAIR - 1))
                nc.tensor.matmul(gB3[:], arenaT[:, 130 * g + 65:130 * g + 130],
                                 arenaT[:, 130 * g + 65:130 * g + 130],
                                 start=(g == 0), stop=(g == NPAIR - 1))
            nc.vector.tensor_copy(gpay(3), gA3[:])
            nc.vector.tensor_tensor(gpay(3), gpay(3), gB3[:], op=ALU.add)

            # ================= layer 3 + staged maxpool =================
            allreduce_G(3)
            stats_and_thresholds(3)
            NT4 = NT // 4            # 512-tiles per stage (stage = 8 k-slices)
            SW = 8 * SLICE           # stage width per half (= NT4 * 512)

            def smx(h, stage):       # stgmax slot [128, SLICE]
                return stgmax[:, (h * 4 + stage) * SLICE:
                              (h * 4 + stage + 1) * SLICE]

            with tc.tile_pool(name="z3p", bufs=2) as z3p:
                for stage in range(4):
                    zs = z3p.tile([128, 2 * SW], BF16, tag="zs")
                    for e in range(NT4):
                        t = stage * NT4 + e
                        for h in range(2):
                            zp3 = pp.tile([128, 512], F32, tag="z")
                            nc.tensor.matmul(
                                zp3[:], wfwd[2][64 * h:64 * h + 64, :],
                                arenaA[64 * h:64 * h + 64,
                                       512 * t:512 * (t + 1)],
                                tile_position=(64 * h, 0))
                            # evac with sign(u3) scale (bf16 ints exact)
                            nc.scalar.activation(
                                zs[:, h * SW + 512 * e:h * SW + 512 * (e + 1)],
                                zp3[:], ACTF.Copy, scale=vv("sgn3"))
                    for h in range(2):
                        tree_max(zs[:, h * SW:(h + 1) * SW], smx(h, stage),
                                 8, SLICE)
                for h in range(2):
                    tree_max(stgmax[:, h * 4 * SLICE:(h + 1) * 4 * SLICE],
                             maxz[:, SLICE * h:SLICE * (h + 1)], 4, SLICE)

            # ---------- final affine + relu ----------
            nc.scalar.activation(y_sb[:], maxz[:], ACTF.Relu,
                                 bias=vv("w3"), scale=vv("absu3"))

            # ---------- transpose to points-major + store ----------
            for j0 in range(0, NB, 4):
                nj = min(4, NB - j0)
                pt = ptr.tile([128, 512], F32, tag="tr")
                for j in range(nj):
                    nc.tensor.transpose(
                        pt[:, 128 * j:128 * (j + 1)],
                        y_sb[:, 128 * (j0 + j):128 * (j0 + j + 1)], idf[:])
                nc.vector.tensor_copy(yt_sb[:, 128 * j0:128 * (j0 + nj)],
                                      pt[:, :128 * nj])
            for j in range(NB):
                h, blk = j // (NB // 2), j % (NB // 2)
                nd = 2 * blk + h
                nc.sync.dma_start(out_dram[128 * nd:128 * (nd + 1), :],
                                  yt_sb[:, 128 * j:128 * (j + 1)])

    nc.compile()
    return nc


def host_inputs(inputs, NB=8, n_cores=NCORES):
    """Build per-core in_maps from full inputs (host-side weight prep)."""
    N = 128 * NB
    NPTS = N * K
    NG = float(n_cores * NPTS)
    bf = DT.np(BF16)
    x = np.asarray(inputs["agg_feat"], np.float32).reshape(-1, D)
    const = {}
    hcvec = {}
    for l in (1, 2, 3):
        W = np.asarray(inputs[f"W{l}"], np.float32)
        s = np.asarray(inputs[f"s{l}"], np.float32)
        g = np.asarray(inputs[f"g{l}"], np.float32)
        b = np.asarray(inputs[f"b{l}"], np.float32)
        Wb = np.sign(W).astype(np.float32)
        Wb[Wb == 0] = 1.0
        cout = W.shape[0]
        wt = Wb.T.astype(np.float32)                    # [cin, cout]
        const[f"wfwd{l}"] = np.vstack([wt, wt]).astype(bf)
        wdup = np.hstack([wt, wt]) if cout == 64 else wt
        const[f"wstat{l}"] = wdup.astype(np.float32)

        def dup(v):
            v = np.asarray(v, np.float32).reshape(-1)
            if v.shape[0] == 64:
                v = np.concatenate([v, v])
            return v

        hcvec[f"gs{l}"] = dup(g * s)
        hcvec[f"beta{l}"] = dup(b)
        hcvec[f"s2{l}"] = dup(s * s)
        if l == 1:
            r = Wb.sum(axis=1)
            hcvec["r41"] = dup(4.0 * r)
            hcvec["rng1"] = dup(NG * r)
            hcvec["rnr1"] = dup(NG * r * r)
    const["hcpack"] = np.stack([hcvec[c] for c in HC_COLS],
                               axis=1).astype(np.float32)
    const["idb"] = np.eye(128, dtype=np.float32).astype(bf)
    const["idf"] = np.eye(128, dtype=np.float32)
    const["ones64"] = np.ones((64, 1), np.float32)

    xr = x.reshape(n_cores, NPTS, D)
    in_maps = []
    for c in range(n_cores):
        m = {"x": np.ascontiguousarray(xr[c])}
        for k2, v in const.items():
            m[k2] = np.ascontiguousarray(v)
        in_maps.append(m)
    return in_maps


def gather_out(results, NB=8, n_cores=NCORES):
    return np.concatenate([np.asarray(results[c]["out"], np.float32)
                           for c in range(n_cores)], axis=0)


_NC_CACHE = {}
LAST_RESULT = None


def kernel(**inputs):
    global LAST_RESULT
    NB = 8
    if NB not in _NC_CACHE:
        _NC_CACHE[NB] = build_nc(NB)
    nc = _NC_CACHE[NB]
    in_maps = host_inputs(inputs, NB)
    trace = bool(int(os.environ.get("KERNEL_TRACE", "0")))
    res = run_bass_kernel_spmd(nc, in_maps, core_ids=list(range(NCORES)),
                               trace=trace)
    LAST_RESULT = res
    return gather_out(res.results, NB)
